# revision 2
# baseline (speedup 1.0000x reference)
"""CopyGenerator kernel for 8x Trainium2 NeuronCores (Bass/Tile).

Strategy: vocab-dim tensor parallel (6250 cols/core).
  logits = hidden @ W.T + b           bf16 matmul on PE, fp32 PSUM accum
  copy   = sigmoid(logits[:, 4])      via exp + reciprocal (no table switch)
  prob   = softmax(logits w/ col4 masked) * (1-copy)
  out    = prob + scatter_add(attn*copy via alignment[src]); out[:,0]=eps
  result = log(out / norm + eps)

Log-domain shortcut: for non-scattered columns
  result[r, v] = logits[r, v] + b[v] + c_r,   c_r = log((1-copy_r)/(Z_r*norm_r))
so the bulk needs NO exp/log/div: just the matmul, a bias add at PSUM drain,
and a per-row scalar add.  exp is only needed to produce Z (row-sum of
exp(logits+b)), done with ACT's fused accum_out.  Z / col0 / col4 logits are
combined across cores with one small (2KB) AllReduce per 128-row tile.
norm is computed analytically:
  norm_r = eps + (1-copy)(1 - e0/Z) + copy*(sum_s attn - attn_mass_into_col0)
Scattered columns (same columns for all t since idx=alignment[src] is
t-independent; rows grouped (b, t) so each 128-row tile has 2 batches) are
recomputed exactly with a tiny extra matmul + exp/log on compacted [128, 32]
tiles; the host places those ~13k values plus columns 0 and 4 into the
assembled output.
"""

import sys

sys.path.insert(0, "/opt/trn_rl_repo")

import numpy as np
import ml_dtypes

import concourse.bass as bass  # noqa: E402
import concourse.mybir as mybir  # noqa: E402
import concourse.tile as tile  # noqa: E402
from concourse import bacc  # noqa: E402
from concourse import bass_utils  # noqa: E402

# ---- problem constants (hardcoded per spec) ----
T, B, S = 64, 32, 50
R = T * B          # 2048 rows
RNN = 512          # hidden size
V = 50000          # vocab
COPY_COL = 4
PAD_COL = 0
EPS = 1e-10
NCORES = 8
VS = V // NCORES   # 6250 cols per core
P = 128            # partitions
KC = RNN // P      # 4 contraction chunks
NT = R // P        # 16 row tiles (b-pairs: tile rt covers b in {2rt, 2rt+1})
NF = 32            # fix-slot columns per row tile
CHUNK = 512
N_CHUNKS = (VS + CHUNK - 1) // CHUNK  # 13 (last = 106)
F32 = mybir.dt.float32
BF16 = mybir.dt.bfloat16

_BUILT = None  # cached (nc, out_names)


def _chunk_sizes():
    out = []
    c = 0
    while c < VS:
        out.append(min(CHUNK, VS - c))
        c += CHUNK
    return out


def build_bass():
    """Build the (single-NEFF, SPMD) bass program. Input-independent."""
    nc = bacc.Bacc(
        "TRN2", target_bir_lowering=False, debug=False, num_devices=NCORES
    )

    # ---- per-core external inputs ----
    hid_in = nc.dram_tensor("hid_t", [P, KC, R], BF16, kind="ExternalInput")
    w_in = nc.dram_tensor("w_t", [P, KC, VS], BF16, kind="ExternalInput")
    b_in = nc.dram_tensor("b_bc", [P, VS], F32, kind="ExternalInput")
    wfix_in = nc.dram_tensor("wfix", [P, NT, KC, NF], BF16, kind="ExternalInput")
    bfix_in = nc.dram_tensor("bfix", [P, NT, NF], F32, kind="ExternalInput")
    asum_in = nc.dram_tensor("asum", [P, NT, NF], F32, kind="ExternalInput")
    a0_in = nc.dram_tensor("a0adj", [P, NT], F32, kind="ExternalInput")
    mask_in = nc.dram_tensor("coremask", [P, 1], F32, kind="ExternalInput")

    # ---- per-core external outputs ----
    bulk_out = nc.dram_tensor("bulk", [R, VS], F32, kind="ExternalOutput")
    fix_out = nc.dram_tensor("fixout", [NT, P, NF], F32, kind="ExternalOutput")
    aux_out = nc.dram_tensor("auxout", [NT, P, 4], F32, kind="ExternalOutput")

    AF = mybir.ActivationFunctionType
    OP = mybir.AluOpType
    rg = [list(range(NCORES))]
    sizes = _chunk_sizes()

    # DRAM view of bulk with rows split as (t, b): row r = t*32 + b.
    # -> [b, t, v] so a (b-pair) tile maps to [2, 64, VS] slices.
    bulk_btv = bulk_out.ap().rearrange("(t b) v -> b t v", t=T, b=B)

    with tile.TileContext(nc) as tc:
        with (
            tc.tile_pool(name="const", bufs=1) as constp,
            tc.tile_pool(name="logits", bufs=3) as logitsp,
            tc.tile_pool(name="esc", bufs=3) as escp,
            tc.tile_pool(name="small", bufs=2) as smallp,
            tc.tile_pool(name="fix", bufs=2) as fixp,
            tc.tile_pool(name="psum", bufs=6, space="PSUM") as psump,
            tc.tile_pool(name="psumf", bufs=2, space="PSUM") as psumfp,
            tc.tile_pool(name="dram", bufs=4, space="DRAM") as dramp,
        ):
            # ---- persistent tiles, loaded once ----
            hid_t = constp.tile([P, KC, R], BF16, tag="hid")
            w_t = constp.tile([P, KC, VS], BF16, tag="w")
            b_t = constp.tile([P, VS], F32, tag="b")
            wfix_t = constp.tile([P, NT, KC, NF], BF16, tag="wfix")
            bfix_t = constp.tile([P, NT, NF], F32, tag="bfix")
            asum_t = constp.tile([P, NT, NF], F32, tag="asum")
            a0_t = constp.tile([P, NT], F32, tag="a0")
            mask_t = constp.tile([P, 1], F32, tag="mask")

            nc.sync.dma_start(hid_t[:], hid_in.ap())
            nc.sync.dma_start(w_t[:], w_in.ap())
            nc.sync.dma_start(b_t[:], b_in.ap())
            nc.gpsimd.dma_start(wfix_t[:], wfix_in.ap())
            nc.gpsimd.dma_start(bfix_t[:], bfix_in.ap())
            nc.gpsimd.dma_start(asum_t[:], asum_in.ap())
            nc.gpsimd.dma_start(a0_t[:], a0_in.ap())
            nc.gpsimd.dma_start(mask_t[:], mask_in.ap())

            for rt in range(NT):
                rows = slice(rt * P, (rt + 1) * P)

                logits = logitsp.tile([P, VS], F32, tag="logits")
                zparts = smallp.tile([P, N_CHUNKS], F32, tag="zparts")

                # --- main matmul + bias-drain + exp/accum per chunk ---
                for j, nj in enumerate(sizes):
                    c0 = j * CHUNK
                    ps = psump.tile([P, CHUNK], F32, tag="ps")
                    for k in range(KC):
                        nc.tensor.matmul(
                            out=ps[:, :nj],
                            lhsT=hid_t[:, k, rows],
                            rhs=w_t[:, k, c0 : c0 + nj],
                            start=(k == 0),
                            stop=(k == KC - 1),
                        )
                    # drain + bias:  logits_chunk = psum + b  (DVE, 1x)
                    nc.vector.tensor_tensor(
                        out=logits[:, c0 : c0 + nj],
                        in0=ps[:, :nj],
                        in1=b_t[:, c0 : c0 + nj],
                        op=OP.add,
                    )
                    # exp for Z with fused row-sum (ACT)
                    ex = escp.tile([P, CHUNK], BF16, tag="ex")
                    nc.scalar.activation(
                        ex[:, :nj],
                        logits[:, c0 : c0 + nj],
                        AF.Exp,
                        accum_out=zparts[:, j : j + 1],
                    )

                # --- fix matmul: recompute logits at scatter-target cols ---
                psf = psumfp.tile([P, NF], F32, tag="psf")
                for k in range(KC):
                    nc.tensor.matmul(
                        out=psf[:],
                        lhsT=hid_t[:, k, rows],
                        rhs=wfix_t[:, rt, k, :],
                        start=(k == 0),
                        stop=(k == KC - 1),
                    )
                lfix = fixp.tile([P, NF], F32, tag="lfix")
                nc.vector.tensor_tensor(
                    out=lfix[:], in0=psf[:], in1=bfix_t[:, rt, :], op=OP.add
                )

                # --- allreduce payload: [Z_partial, l0*m, l4*m, 0] ---
                zin = smallp.tile([P, 4], F32, tag="zin")
                nc.vector.tensor_reduce(
                    out=zin[:, 0:1],
                    in_=zparts[:, 0:N_CHUNKS],
                    axis=mybir.AxisListType.X,
                    op=OP.add,
                )
                nc.vector.tensor_scalar(
                    out=zin[:, 1:2], in0=logits[:, PAD_COL : PAD_COL + 1],
                    scalar1=mask_t[:, 0:1], scalar2=None, op0=OP.mult,
                )
                nc.vector.tensor_scalar(
                    out=zin[:, 2:3], in0=logits[:, COPY_COL : COPY_COL + 1],
                    scalar1=mask_t[:, 0:1], scalar2=None, op0=OP.mult,
                )
                nc.vector.tensor_scalar(
                    out=zin[:, 3:4], in0=mask_t[:, 0:1],
                    scalar1=0.0, scalar2=None, op0=OP.mult,
                )
                zin_d = dramp.tile([P, 4], F32, tag="zin_d")
                zout_d = dramp.tile([P, 4], F32, tag="zout_d")
                nc.gpsimd.dma_start(zin_d[:], zin[:])
                nc.gpsimd.collective_compute(
                    "AllReduce",
                    OP.add,
                    replica_groups=rg,
                    ins=[zin_d[:].opt()],
                    outs=[zout_d[:].opt()],
                )
                zout = smallp.tile([P, 4], F32, tag="zout")
                nc.gpsimd.dma_start(zout[:], zout_d[:])

                # --- per-row scalar chain ([P,1] ops) ---
                # scal cols: 0:e4 1:tmp 2:em4 3:e0 4:e0z 5:v1 6:v2 7:v3
                #            8:Z 9:copy 10:norm 11:c 12:omc 13:invZ
                #            14:invnorm 15:s1a 16:s1 17:s2
                sc = smallp.tile([P, 18], F32, tag="scal")
                zr = zout[:, 0:1]
                l0b = zout[:, 1:2]
                l4b = zout[:, 2:3]
                e4, tmp, em4, e0 = sc[:, 0:1], sc[:, 1:2], sc[:, 2:3], sc[:, 3:4]
                e0z, v1, v2, v3 = sc[:, 4:5], sc[:, 5:6], sc[:, 6:7], sc[:, 7:8]
                Zc, cpy, nrm, cc = sc[:, 8:9], sc[:, 9:10], sc[:, 10:11], sc[:, 11:12]
                omc, invZ, invn = sc[:, 12:13], sc[:, 13:14], sc[:, 14:15]
                s1a, s1, s2 = sc[:, 15:16], sc[:, 16:17], sc[:, 17:18]

                nc.scalar.activation(e4, l4b, AF.Exp)                    # e^{l4}
                nc.vector.tensor_tensor(out=Zc, in0=zr, in1=e4, op=OP.subtract)
                nc.scalar.activation(em4, l4b, AF.Exp, scale=-1.0)       # e^{-l4}
                nc.vector.tensor_scalar(
                    out=tmp, in0=em4, scalar1=1.0, scalar2=None, op0=OP.add
                )
                nc.vector.reciprocal(out=cpy, in_=tmp)                   # sigmoid
                nc.vector.tensor_scalar(
                    out=omc, in0=cpy, scalar1=-1.0, scalar2=1.0,
                    op0=OP.mult, op1=OP.add,
                )                                                        # 1-copy
                nc.vector.reciprocal(out=invZ, in_=Zc)
                nc.scalar.activation(e0, l0b, AF.Exp)
                nc.vector.tensor_scalar(
                    out=e0z, in0=e0, scalar1=invZ, scalar2=None, op0=OP.mult
                )
                nc.vector.tensor_scalar(
                    out=v1, in0=e0z, scalar1=-1.0, scalar2=1.0,
                    op0=OP.mult, op1=OP.add,
                )                                                        # 1-e0/Z
                nc.vector.tensor_scalar(
                    out=v2, in0=v1, scalar1=omc, scalar2=None, op0=OP.mult
                )
                nc.vector.tensor_scalar(
                    out=v3, in0=a0_t[:, rt : rt + 1], scalar1=cpy,
                    scalar2=None, op0=OP.mult,
                )                                                        # copy*(sa-a0)
                nc.vector.tensor_scalar(
                    out=nrm, in0=v2, scalar1=v3, scalar2=EPS,
                    op0=OP.add, op1=OP.add,
                )                                                        # norm
                nc.vector.reciprocal(out=invn, in_=nrm)
                nc.vector.tensor_scalar(
                    out=s1a, in0=omc, scalar1=invZ, scalar2=None, op0=OP.mult
                )
                nc.vector.tensor_scalar(
                    out=s1, in0=s1a, scalar1=invn, scalar2=None, op0=OP.mult
                )                                                        # (1-c)/(Z n)
                nc.scalar.activation(cc, s1, AF.Ln)                      # c_r
                nc.vector.tensor_scalar(
                    out=s2, in0=cpy, scalar1=invn, scalar2=None, op0=OP.mult
                )

                # aux export: [Z, copy, norm, c] = sc cols 8..11
                nc.gpsimd.dma_start(aux_out.ap()[rt], sc[:, 8:12])

                # --- finish fix columns exactly ---
                efix = fixp.tile([P, NF], F32, tag="efix")
                nc.scalar.activation(efix[:], lfix[:], AF.Exp)
                t1 = fixp.tile([P, NF], F32, tag="t1")
                nc.vector.tensor_scalar(
                    out=t1[:], in0=efix[:], scalar1=s1, scalar2=None, op0=OP.mult
                )
                t2 = fixp.tile([P, NF], F32, tag="t2")
                nc.vector.tensor_scalar(
                    out=t2[:], in0=asum_t[:, rt, :], scalar1=s2, scalar2=EPS,
                    op0=OP.mult, op1=OP.add,
                )
                vfix = fixp.tile([P, NF], F32, tag="vfix")
                nc.vector.tensor_tensor(out=vfix[:], in0=t1[:], in1=t2[:], op=OP.add)
                ffix = fixp.tile([P, NF], F32, tag="ffix")
                nc.scalar.activation(ffix[:], vfix[:], AF.Ln)
                nc.gpsimd.dma_start(fix_out.ap()[rt], ffix[:])

                # --- bulk: logits += c_r, then DMA out (strided to (t,b)) ---
                for j, nj in enumerate(sizes):
                    c0 = j * CHUNK
                    nc.vector.tensor_scalar(
                        out=logits[:, c0 : c0 + nj],
                        in0=logits[:, c0 : c0 + nj],
                        scalar1=cc, scalar2=None, op0=OP.add,
                    )
                # 4 output DMAs per tile (stripe columns across queues)
                spans = [(0, 1664), (1664, 3328), (3328, 4992), (4992, VS)]
                for lo, hi in spans:
                    nc.sync.dma_start(
                        bulk_btv[2 * rt : 2 * rt + 2, :, lo:hi],
                        logits[:, lo:hi],
                    )

    nc.compile()
    return nc


# ---------------- host side ----------------

def _prep_inputs(hidden, attn, W, b, src, alignment):
    """Build per-core in_maps + metadata for host-side assembly."""
    hidden = np.asarray(hidden, np.float32)
    attn = np.asarray(attn, np.float32)
    W = np.asarray(W, np.float32)
    b = np.asarray(b, np.float32)
    src = np.asarray(src)
    alignment = np.asarray(alignment)

    idx = alignment[src]  # (B, S) target vocab ids, t-independent
    idx = np.asarray(idx, np.int64)

    # rows reordered (t,b) -> (b,t):  row_bt = b*T + t
    hid_bt = hidden.reshape(T, B, RNN).transpose(1, 0, 2).reshape(R, RNN)
    attn_bt = attn.reshape(T, B, S).transpose(1, 0, 2).reshape(R, S)

    hid_t = np.ascontiguousarray(
        hid_bt.T.reshape(KC, P, R).transpose(1, 0, 2)
    ).astype(ml_dtypes.bfloat16)  # [P, KC, R]

    attnsum = attn_bt.sum(axis=1)  # [R] in (b,t) order
    # attention mass into col 0 per (b,t) row
    a0 = np.zeros(R, np.float32)
    for bb in range(B):
        s_hit = np.nonzero(idx[bb] == PAD_COL)[0]
        if len(s_hit):
            a0[bb * T : (bb + 1) * T] = attn_bt[
                bb * T : (bb + 1) * T, s_hit
            ].sum(axis=1)
    a0adj = (attnsum - a0).reshape(NT, P).T.copy()  # [P, NT]

    # per-(core, b) scatter targets, excluding cols 0 and 4
    # fix slot tables per core
    per_core = []
    for k in range(NCORES):
        c0, c1 = k * VS, (k + 1) * VS
        wk = np.ascontiguousarray(
            W[c0:c1].T.reshape(KC, P, VS).transpose(1, 0, 2)
        ).astype(ml_dtypes.bfloat16)  # [P, KC, VS]
        bk = np.broadcast_to(b[c0:c1], (P, VS)).astype(np.float32)

        wfix = np.zeros((P, NT, KC, NF), ml_dtypes.bfloat16)
        bfix = np.zeros((P, NT, NF), np.float32)
        asum = np.zeros((P, NT, NF), np.float32)
        slots = []  # list per rt of list of (b, v_global, slot_j)
        overflow = []  # (b, v_global) handled on host
        Wb = W.astype(ml_dtypes.bfloat16).astype(np.float32)  # n/a, placeholder
        for rt in range(NT):
            tile_slots = []
            jslot = 0
            for bl in range(2):
                bb = 2 * rt + bl
                vs_here, counts = np.unique(idx[bb], return_counts=True)
                for v in vs_here:
                    if not (c0 <= v < c1) or v == PAD_COL or v == COPY_COL:
                        continue
                    if jslot >= NF:
                        overflow.append((bb, int(v)))
                        continue
                    vl = int(v - c0)
                    wcol = W[v]  # (RNN,)
                    wfix[:, rt, :, jslot] = (
                        wcol.reshape(KC, P).T.astype(ml_dtypes.bfloat16)
                    )
                    bfix[:, rt, jslot] = b[v]
                    s_hit = np.nonzero(idx[bb] == v)[0]
                    rowsl = slice(bl * T, (bl + 1) * T)
                    asum[rowsl, rt, jslot] = attn_bt[
                        bb * T : (bb + 1) * T][:, s_hit].sum(axis=1)
                    tile_slots.append((bb, int(v), jslot))
                    jslot += 1
            slots.append(tile_slots)

        mask = np.full((P, 1), 1.0 if k == 0 else 0.0, np.float32)
        in_map = {
            "hid_t": np.asarray(hid_t),
            "w_t": np.asarray(wk),
            "b_bc": bk,
            "wfix": np.asarray(wfix),
            "bfix": bfix,
            "asum": asum,
            "a0adj": a0adj,
            "coremask": mask,
        }
        per_core.append((in_map, slots, overflow))

    meta = {
        "idx": idx,
        "attn": attn,
        "hidden": hidden,
        "W": W,
        "b": b,
    }
    return per_core, meta


def _assemble(results, per_core, meta):
    idx = meta["idx"]
    attn = meta["attn"]
    full = np.empty((R, V), np.float32)
    for k in range(NCORES):
        full[:, k * VS : (k + 1) * VS] = results[k]["bulk"]

    # aux from core 0: [NT, P, 4] = Z, copy, norm, c   (rows in (b,t) order)
    aux = results[0]["auxout"].reshape(R, 4)  # row_bt = rt*128+p = b*T+t
    # reorder to original rows r = t*B + b
    bt_of_r = (np.arange(R) % B) * T + (np.arange(R) // B)
    Z = aux[bt_of_r, 0]
    copy = aux[bt_of_r, 1]
    norm = aux[bt_of_r, 2]

    # scatter-fix placement
    for k in range(NCORES):
        fixv = results[k]["fixout"]  # [NT, P, NF], rows (b,t)
        _, slots, overflow = per_core[k]
        for rt in range(NT):
            for (bb, v, j) in slots[rt]:
                bl = bb - 2 * rt
                rr = np.arange(T) * B + bb
                full[rr, v] = fixv[rt, bl * T : (bl + 1) * T, j]
        for (bb, v) in overflow:
            # exact host recompute for rare overflow targets
            rr = np.arange(T) * B + bb
            hid = meta["hidden"][rr]
            l = hid.astype(np.float32) @ meta["W"][v] + meta["b"][v]
            s_hit = np.nonzero(idx[bb] == v)[0]
            am = attn[rr][:, s_hit].sum(axis=1)
            pv = np.exp(l) * (1.0 - copy[rr]) / Z[rr]
            full[rr, v] = np.log((pv + copy[rr] * am) / norm[rr] + EPS)

    # columns 0 (PAD) and 4 (COPY, masked from softmax)
    full[:, PAD_COL] = np.log(EPS / norm + EPS)
    am4 = np.zeros(R, np.float32)
    for bb in range(B):
        s_hit = np.nonzero(idx[bb] == COPY_COL)[0]
        if len(s_hit):
            rr = np.arange(T) * B + bb
            am4[rr] = attn[rr][:, s_hit].sum(axis=1)
    full[:, COPY_COL] = np.log(copy * am4 / norm + EPS)
    return full


def _get_built():
    global _BUILT
    if _BUILT is None:
        _BUILT = build_bass()
    return _BUILT


def run(inputs, trace=False):
    nc = _get_built()
    per_core, meta = _prep_inputs(**inputs)
    in_maps = [pc[0] for pc in per_core]
    res = bass_utils.run_bass_kernel_spmd(
        nc, in_maps, core_ids=list(range(NCORES)), trace=trace
    )
    out = _assemble(res.results, per_core, meta)
    return out, res


def kernel(**inputs) -> np.ndarray:
    out, _ = run(inputs, trace=False)
    return out
